# revision 2
# baseline (speedup 1.0000x reference)
"""BiLSTM (B=64, T=512, D_IN=512, H=1024) on 8 TRN2 NeuronCores — v6.

One merged f/b AllGather per timestep (bf16 [128, 2B] payload — measured
5.9us each, the lowest-collective-floor structure) with the serial per-step
chain cut down:
- x-projection (+bias) computed 2 steps ahead into SBUF (off the chain,
  PE filler during collective windows),
- gathered h^T read back in 4 wide DMAs (instruction issue on the Sync
  queue is ~0.6us each — fewer, bigger transfers),
- gate columns reordered host-side to (i, f, o, g) so one ACT instruction
  applies sigmoid to all three sigmoid gates,
- matmuls bf16 (fp32 PSUM accumulate, fp32 cell state).
"""

import sys

if "/opt/trn_rl_repo" not in sys.path:
    sys.path.insert(0, "/opt/trn_rl_repo")

from contextlib import ExitStack

import numpy as np

B, T, D_IN, H, D_OUT = 64, 512, 512, 1024, 512
NC_N = 8
HJ = H // NC_N  # 128 — per-core H slice
GJ = 4 * HJ  # 512 — per-core gate columns (i|f|o|g, 128 each)
KD = D_IN // 128  # 4 k-chunks over D_IN
KH = H // 128  # 8 k-chunks over H
LOOK = 2  # x-projection lookahead steps


def build(t_steps=T):
    import concourse.mybir as mybir
    import concourse.tile as tile
    from concourse import bacc
    from concourse.masks import make_identity

    f32 = mybir.dt.float32
    bf16 = mybir.dt.bfloat16
    AF = mybir.ActivationFunctionType

    nc = bacc.Bacc(None, target_bir_lowering=False, num_devices=NC_N)

    xT = nc.dram_tensor("xT", [t_steps, D_IN, B], bf16, kind="ExternalInput")
    wih = {}
    whh = {}
    bias = {}
    for d in "fb":
        wih[d] = nc.dram_tensor(f"wihT_{d}", [D_IN, GJ], bf16, kind="ExternalInput")
        whh[d] = nc.dram_tensor(f"whhT_{d}", [H, GJ], bf16, kind="ExternalInput")
        bias[d] = nc.dram_tensor(f"bias_{d}", [B, GJ], f32, kind="ExternalInput")
    wlin = nc.dram_tensor("wlinT", [2 * H, D_OUT], bf16, kind="ExternalInput")
    blin = nc.dram_tensor("blin", [B, D_OUT], f32, kind="ExternalInput")
    out = nc.dram_tensor("out", [B, D_OUT], f32, kind="ExternalOutput")

    with ExitStack() as ctx:
        tc = ctx.enter_context(tile.TileContext(nc))
        const = ctx.enter_context(tc.tile_pool(name="const", bufs=1))
        state = ctx.enter_context(tc.tile_pool(name="state", bufs=1))
        xpool = ctx.enter_context(tc.tile_pool(name="xpool", bufs=8))
        xsp = ctx.enter_context(tc.tile_pool(name="xsp", bufs=8))
        work = ctx.enter_context(tc.tile_pool(name="work", bufs=4))
        pg = ctx.enter_context(tc.tile_pool(name="pg", bufs=2, space="PSUM"))
        pgx = ctx.enter_context(tc.tile_pool(name="pgx", bufs=2, space="PSUM"))
        po = ctx.enter_context(tc.tile_pool(name="po", bufs=1, space="PSUM"))
        ptr = ctx.enter_context(tc.tile_pool(name="ptr", bufs=2, space="PSUM"))
        dram = ctx.enter_context(tc.tile_pool(name="dram", bufs=2, space="DRAM"))

        wih_sb = {}
        whh_sb = {}
        bias_sb = {}
        for d in "fb":
            wih_sb[d] = const.tile([128, KD, GJ], bf16, name=f"wih_sb_{d}")
            nc.sync.dma_start(
                wih_sb[d][:], wih[d][:].rearrange("(c p) n -> p c n", p=128)
            )
            whh_sb[d] = const.tile([128, KH, GJ], bf16, name=f"whh_sb_{d}")
            nc.sync.dma_start(
                whh_sb[d][:], whh[d][:].rearrange("(c p) n -> p c n", p=128)
            )
            bias_sb[d] = const.tile([B, GJ], f32, name=f"bias_sb_{d}")
            nc.sync.dma_start(bias_sb[d][:], bias[d][:])
        wlin_sb = const.tile([128, 2 * KH, D_OUT], bf16)
        nc.sync.dma_start(wlin_sb[:], wlin[:].rearrange("(c p) n -> p c n", p=128))
        blin_sb = const.tile([B, D_OUT], f32)
        nc.sync.dma_start(blin_sb[:], blin[:])
        ident = const.tile([B, B], f32)
        make_identity(nc, ident[:])

        # hT[:, c, 0:B] = forward h chunk c, hT[:, c, B:2B] = backward.
        hT = state.tile([128, KH, 2 * B], bf16, name="hT")
        nc.vector.memset(hT[:].bitcast(mybir.dt.uint16), 0.0)
        c_st = {}
        for d in "fb":
            c_st[d] = state.tile([B, HJ], f32, name=f"c_{d}")
            nc.vector.memset(c_st[d][:], 0.0)

        xpre = {}

        def produce_xpre(d, t):
            t_eff = t if d == "f" else t_steps - 1 - t
            xt = xpool.tile([128, KD, B], bf16, tag="xt", name=f"xt_{d}{t}")
            nc.sync.dma_start(xt[:], xT[t_eff].rearrange("(c p) b -> p c b", p=128))
            x_ps = pgx.tile([B, GJ], f32, tag="x_ps", name=f"xps_{d}{t}")
            for k in range(KD):
                nc.tensor.matmul(
                    x_ps[:],
                    xt[:, k, :],
                    wih_sb[d][:, k, :],
                    start=(k == 0),
                    stop=(k == KD - 1),
                )
            xp = xsp.tile([B, GJ], f32, tag=f"xs_{d}", name=f"xp_{d}{t}")
            nc.vector.tensor_add(xp[:], x_ps[:], bias_sb[d][:])
            xpre[(d, t)] = xp

        for d in "fb":
            for tt in range(min(LOOK, t_steps)):
                produce_xpre(d, tt)

        for t in range(t_steps):
            comb = work.tile([HJ, 2 * B], bf16, tag="comb", name=f"comb{t}")
            for di, d in enumerate("fb"):
                if t + LOOK < t_steps:
                    produce_xpre(d, t + LOOK)
                g_ps = pg.tile([B, GJ], f32, tag="g_ps", name=f"g_ps_{d}{t}")
                for k in range(KH):
                    nc.tensor.matmul(
                        g_ps[:],
                        hT[:, k, di * B : (di + 1) * B],
                        whh_sb[d][:, k, :],
                        start=(k == 0),
                        stop=(k == KH - 1),
                    )
                pre = work.tile([B, GJ], f32, tag="pre", name=f"pre_{d}{t}")
                nc.vector.tensor_add(pre[:], g_ps[:], xpre.pop((d, t))[:])
                acts = work.tile([B, GJ], f32, tag="acts", name=f"acts_{d}{t}")
                # gate order is (i, f, o, g): one sigmoid over 3 gates + tanh.
                nc.scalar.activation(acts[:, 0 : 3 * HJ], pre[:, 0 : 3 * HJ], AF.Sigmoid)
                nc.scalar.activation(acts[:, 3 * HJ : 4 * HJ], pre[:, 3 * HJ : 4 * HJ], AF.Tanh)
                ig = work.tile([B, HJ], f32, tag="ig", name=f"ig_{d}{t}")
                fc = work.tile([B, HJ], f32, tag="fc", name=f"fc_{d}{t}")
                nc.vector.tensor_mul(ig[:], acts[:, 0:HJ], acts[:, 3 * HJ : 4 * HJ])
                nc.vector.tensor_mul(fc[:], acts[:, HJ : 2 * HJ], c_st[d][:])
                nc.vector.tensor_add(c_st[d][:], ig[:], fc[:])
                tnh = work.tile([B, HJ], f32, tag="tnh", name=f"tnh_{d}{t}")
                nc.scalar.activation(tnh[:], c_st[d][:], AF.Tanh)
                hnew = work.tile([B, HJ], f32, tag="hnew", name=f"hnew_{d}{t}")
                nc.vector.tensor_mul(hnew[:], acts[:, 2 * HJ : 3 * HJ], tnh[:])
                tr_ps = ptr.tile([HJ, B], f32, tag="tr_ps", name=f"tr_{d}{t}")
                nc.tensor.transpose(tr_ps[:], hnew[:], ident[:])
                nc.vector.tensor_copy(comb[:, di * B : (di + 1) * B], tr_ps[:])

            ag_i = dram.tile([HJ, 2 * B], bf16, tag="ag_i", name=f"agi{t}")
            ag_o = dram.tile(
                [H, 2 * B], bf16, tag="ag_o", name=f"ago{t}", addr_space="Shared"
            )
            nc.sync.dma_start(ag_i[:], comb[:])
            nc.gpsimd.collective_compute(
                "AllGather",
                mybir.AluOpType.bypass,
                replica_groups=[list(range(NC_N))],
                ins=[ag_i[:].opt()],
                outs=[ag_o[:].opt()],
            )
            for c in range(4):
                nc.sync.dma_start(
                    hT[:, 2 * c : 2 * c + 2, :],
                    ag_o[c * 256 : (c + 1) * 256, :].rearrange(
                        "(c p) b -> p c b", p=128
                    ),
                )

        o_ps = po.tile([B, D_OUT], f32, tag="o_ps", bufs=1)
        for k in range(KH):
            nc.tensor.matmul(
                o_ps[:],
                hT[:, k, 0:B],
                wlin_sb[:, k, :],
                start=(k == 0),
                stop=False,
            )
        for k in range(KH):
            nc.tensor.matmul(
                o_ps[:],
                hT[:, k, B : 2 * B],
                wlin_sb[:, KH + k, :],
                start=False,
                stop=(k == KH - 1),
            )
        o_sb = work.tile([B, D_OUT], f32, tag="o_sb")
        nc.vector.tensor_add(o_sb[:], o_ps[:], blin_sb[:])
        nc.sync.dma_start(out[:], o_sb[:])
    nc.compile()
    return nc


def make_in_maps(
    x, W_ih_f, W_hh_f, b_ih_f, b_hh_f, W_ih_b, W_hh_b, b_ih_b, b_hh_b, W_lin, b_lin
):
    import ml_dtypes

    bf = ml_dtypes.bfloat16
    t_steps = x.shape[1]
    xTs = np.ascontiguousarray(
        np.asarray(x, np.float32).transpose(1, 2, 0).astype(bf)
    )
    W = {
        "f": (np.asarray(W_ih_f, np.float32), np.asarray(W_hh_f, np.float32),
              np.asarray(b_ih_f, np.float32) + np.asarray(b_hh_f, np.float32)),
        "b": (np.asarray(W_ih_b, np.float32), np.asarray(W_hh_b, np.float32),
              np.asarray(b_ih_b, np.float32) + np.asarray(b_hh_b, np.float32)),
    }
    wlinT = np.ascontiguousarray(np.asarray(W_lin, np.float32).T.astype(bf))
    blin_rep = np.broadcast_to(np.asarray(b_lin, np.float32), (B, D_OUT)).copy()
    in_maps = []
    # PyTorch gate blocks in weight rows: i, f, g, o. Our column order is
    # (i, f, o, g) so the three sigmoid gates are contiguous.
    gate_rows = (0, 1, 3, 2)
    for j in range(NC_N):
        m = {"xT": xTs, "wlinT": wlinT, "blin": blin_rep}
        cols = np.concatenate(
            [np.arange(g * H + j * HJ, g * H + (j + 1) * HJ) for g in gate_rows]
        )
        for d in "fb":
            W_ih, W_hh, b_sum = W[d]
            m[f"wihT_{d}"] = np.ascontiguousarray(W_ih.T[:, cols].astype(bf))
            m[f"whhT_{d}"] = np.ascontiguousarray(W_hh.T[:, cols].astype(bf))
            m[f"bias_{d}"] = np.broadcast_to(b_sum[cols], (B, GJ)).copy()
        in_maps.append(m)
    return in_maps


def kernel(**inputs) -> np.ndarray:
    from concourse.bass_utils import run_bass_kernel_spmd

    in_maps = make_in_maps(**inputs)
    nc = build(inputs["x"].shape[1])
    res = run_bass_kernel_spmd(nc, in_maps, core_ids=list(range(NC_N)))
    return res.results[0]["out"]


# revision 3
# speedup vs baseline: 1.0665x; 1.0665x over previous
"""BiLSTM (B=64, T=512, D_IN=512, H=1024) on 8 TRN2 NeuronCores — v8.

v6 + direction stacking: forward occupies partitions 0-63, backward 64-127
throughout the cell. The f/b gate matmuls are issued as column-tiled pairs
(tile_position (0,0) / (0,64)) that run CONCURRENTLY in the two halves of
the PE array, each streaming its own direction's weights — halving the
PE-serial h-matmul block. All elementwise/activation work runs once on
[128, *] tiles (full DVE/ACT lanes) instead of twice on [64, *].
"""

import sys

if "/opt/trn_rl_repo" not in sys.path:
    sys.path.insert(0, "/opt/trn_rl_repo")

from contextlib import ExitStack

import numpy as np

B, T, D_IN, H, D_OUT = 64, 512, 512, 1024, 512
NC_N = 8
HJ = H // NC_N  # 128 — per-core H slice
GJ = 4 * HJ  # 512 — per-core gate columns (i|f|o|g, 128 each)
KD = D_IN // 128  # 4 k-chunks over D_IN
KH = H // 128  # 8 k-chunks over H
LOOK = 2  # x-projection lookahead steps


def build(t_steps=T):
    import concourse.mybir as mybir
    import concourse.tile as tile
    from concourse import bacc
    from concourse.masks import make_identity

    f32 = mybir.dt.float32
    bf16 = mybir.dt.bfloat16
    AF = mybir.ActivationFunctionType

    nc = bacc.Bacc(None, target_bir_lowering=False, num_devices=NC_N)

    xT = nc.dram_tensor("xT", [t_steps, D_IN, B], bf16, kind="ExternalInput")
    wih = {}
    whh = {}
    for d in "fb":
        wih[d] = nc.dram_tensor(f"wihT_{d}", [D_IN, GJ], bf16, kind="ExternalInput")
        whh[d] = nc.dram_tensor(f"whhT_{d}", [H, GJ], bf16, kind="ExternalInput")
    bias_fb = nc.dram_tensor("bias_fb", [2 * B, GJ], f32, kind="ExternalInput")
    wlin = nc.dram_tensor("wlinT", [2 * H, D_OUT], bf16, kind="ExternalInput")
    blin = nc.dram_tensor("blin", [B, D_OUT], f32, kind="ExternalInput")
    out = nc.dram_tensor("out", [B, D_OUT], f32, kind="ExternalOutput")

    with ExitStack() as ctx:
        tc = ctx.enter_context(tile.TileContext(nc))
        const = ctx.enter_context(tc.tile_pool(name="const", bufs=1))
        state = ctx.enter_context(tc.tile_pool(name="state", bufs=1))
        xpool = ctx.enter_context(tc.tile_pool(name="xpool", bufs=8))
        xsp = ctx.enter_context(tc.tile_pool(name="xsp", bufs=4))
        work = ctx.enter_context(tc.tile_pool(name="work", bufs=4))
        pg = ctx.enter_context(tc.tile_pool(name="pg", bufs=2, space="PSUM"))
        pgx = ctx.enter_context(tc.tile_pool(name="pgx", bufs=2, space="PSUM"))
        po = ctx.enter_context(tc.tile_pool(name="po", bufs=1, space="PSUM"))
        ptr = ctx.enter_context(tc.tile_pool(name="ptr", bufs=2, space="PSUM"))
        dram = ctx.enter_context(tc.tile_pool(name="dram", bufs=2, space="DRAM"))

        wih_sb = {}
        whh_sb = {}
        for d in "fb":
            wih_sb[d] = const.tile([128, KD, GJ], bf16, name=f"wih_sb_{d}")
            nc.sync.dma_start(
                wih_sb[d][:], wih[d][:].rearrange("(c p) n -> p c n", p=128)
            )
            whh_sb[d] = const.tile([128, KH, GJ], bf16, name=f"whh_sb_{d}")
            nc.sync.dma_start(
                whh_sb[d][:], whh[d][:].rearrange("(c p) n -> p c n", p=128)
            )
        bias_sb = const.tile([2 * B, GJ], f32, name="bias_sb")
        nc.sync.dma_start(bias_sb[:], bias_fb[:])
        wlin_sb = const.tile([128, 2 * KH, D_OUT], bf16)
        nc.sync.dma_start(wlin_sb[:], wlin[:].rearrange("(c p) n -> p c n", p=128))
        blin_sb = const.tile([B, D_OUT], f32)
        nc.sync.dma_start(blin_sb[:], blin[:])
        ident = const.tile([B, B], f32)
        make_identity(nc, ident[:])

        # hT[:, c, 0:B] = forward h chunk c, hT[:, c, B:2B] = backward.
        hT = state.tile([128, KH, 2 * B], bf16, name="hT")
        nc.vector.memset(hT[:].bitcast(mybir.dt.uint16), 0.0)
        # cell state stacked: rows 0:B forward, B:2B backward
        c_st = state.tile([2 * B, HJ], f32, name="c_st")
        nc.vector.memset(c_st[:], 0.0)

        xpre = {}

        def produce_xpre(t):
            xts = {}
            for d, t_eff in (("f", t), ("b", t_steps - 1 - t)):
                xts[d] = xpool.tile([128, KD, B], bf16, tag=f"xt_{d}", name=f"xt_{d}{t}")
                nc.sync.dma_start(
                    xts[d][:], xT[t_eff].rearrange("(c p) b -> p c b", p=128)
                )
            x_ps = pgx.tile([2 * B, GJ], f32, tag="x_ps", name=f"xps_{t}")
            for k in range(KD):
                nc.tensor.matmul(
                    x_ps[0:B, :],
                    xts["f"][:, k, :],
                    wih_sb["f"][:, k, :],
                    start=(k == 0),
                    stop=(k == KD - 1),
                    tile_position=(0, 0),
                    skip_group_check=True,
                )
                nc.tensor.matmul(
                    x_ps[B : 2 * B, :],
                    xts["b"][:, k, :],
                    wih_sb["b"][:, k, :],
                    start=(k == 0),
                    stop=(k == KD - 1),
                    tile_position=(0, 64),
                    skip_group_check=True,
                )
            xp = xsp.tile([2 * B, GJ], f32, tag="xs", name=f"xp_{t}")
            nc.vector.tensor_add(xp[:], x_ps[:], bias_sb[:])
            xpre[t] = xp

        for tt in range(min(LOOK, t_steps)):
            produce_xpre(tt)

        for t in range(t_steps):
            if t + LOOK < t_steps:
                produce_xpre(t + LOOK)
            g_ps = pg.tile([2 * B, GJ], f32, tag="g_ps", name=f"g_ps_{t}")
            for k in range(KH):
                nc.tensor.matmul(
                    g_ps[0:B, :],
                    hT[:, k, 0:B],
                    whh_sb["f"][:, k, :],
                    start=(k == 0),
                    stop=(k == KH - 1),
                    tile_position=(0, 0),
                    skip_group_check=True,
                )
                nc.tensor.matmul(
                    g_ps[B : 2 * B, :],
                    hT[:, k, B : 2 * B],
                    whh_sb["b"][:, k, :],
                    start=(k == 0),
                    stop=(k == KH - 1),
                    tile_position=(0, 64),
                    skip_group_check=True,
                )
            pre = work.tile([2 * B, GJ], f32, tag="pre", name=f"pre_{t}")
            nc.vector.tensor_add(pre[:], g_ps[:], xpre.pop(t)[:])
            acts = work.tile([2 * B, GJ], f32, tag="acts", name=f"acts_{t}")
            # gate order is (i, f, o, g): one sigmoid over 3 gates + tanh.
            nc.scalar.activation(acts[:, 0 : 3 * HJ], pre[:, 0 : 3 * HJ], AF.Sigmoid)
            nc.scalar.activation(acts[:, 3 * HJ : 4 * HJ], pre[:, 3 * HJ : 4 * HJ], AF.Tanh)
            ig = work.tile([2 * B, HJ], f32, tag="ig", name=f"ig_{t}")
            fc = work.tile([2 * B, HJ], f32, tag="fc", name=f"fc_{t}")
            nc.vector.tensor_mul(ig[:], acts[:, 0:HJ], acts[:, 3 * HJ : 4 * HJ])
            nc.vector.tensor_mul(fc[:], acts[:, HJ : 2 * HJ], c_st[:])
            nc.vector.tensor_add(c_st[:], ig[:], fc[:])
            tnh = work.tile([2 * B, HJ], f32, tag="tnh", name=f"tnh_{t}")
            nc.scalar.activation(tnh[:], c_st[:], AF.Tanh)
            comb = work.tile([HJ, 2 * B], bf16, tag="comb", name=f"comb{t}")
            for di in range(2):
                hnew = work.tile([B, HJ], f32, tag=f"hnew{di}", name=f"hnew_{di}{t}")
                nc.vector.tensor_mul(
                    hnew[:],
                    acts[di * B : (di + 1) * B, 2 * HJ : 3 * HJ],
                    tnh[di * B : (di + 1) * B, :],
                )
                tr_ps = ptr.tile([HJ, B], f32, tag="tr_ps", name=f"tr_{di}{t}")
                nc.tensor.transpose(tr_ps[:], hnew[:], ident[:])
                nc.vector.tensor_copy(comb[:, di * B : (di + 1) * B], tr_ps[:])

            ag_i = dram.tile([HJ, 2 * B], bf16, tag="ag_i", name=f"agi{t}")
            ag_o = dram.tile(
                [H, 2 * B], bf16, tag="ag_o", name=f"ago{t}", addr_space="Shared"
            )
            nc.sync.dma_start(ag_i[:], comb[:])
            nc.gpsimd.collective_compute(
                "AllGather",
                mybir.AluOpType.bypass,
                replica_groups=[list(range(NC_N))],
                ins=[ag_i[:].opt()],
                outs=[ag_o[:].opt()],
            )
            for c in range(4):
                nc.sync.dma_start(
                    hT[:, 2 * c : 2 * c + 2, :],
                    ag_o[c * 256 : (c + 1) * 256, :].rearrange(
                        "(c p) b -> p c b", p=128
                    ),
                )

        o_ps = po.tile([B, D_OUT], f32, tag="o_ps", bufs=1)
        for k in range(KH):
            nc.tensor.matmul(
                o_ps[:],
                hT[:, k, 0:B],
                wlin_sb[:, k, :],
                start=(k == 0),
                stop=False,
            )
        for k in range(KH):
            nc.tensor.matmul(
                o_ps[:],
                hT[:, k, B : 2 * B],
                wlin_sb[:, KH + k, :],
                start=False,
                stop=(k == KH - 1),
            )
        o_sb = work.tile([B, D_OUT], f32, tag="o_sb")
        nc.vector.tensor_add(o_sb[:], o_ps[:], blin_sb[:])
        nc.sync.dma_start(out[:], o_sb[:])
    nc.compile()
    return nc


def make_in_maps(
    x, W_ih_f, W_hh_f, b_ih_f, b_hh_f, W_ih_b, W_hh_b, b_ih_b, b_hh_b, W_lin, b_lin
):
    import ml_dtypes

    bf = ml_dtypes.bfloat16
    t_steps = x.shape[1]
    xTs = np.ascontiguousarray(
        np.asarray(x, np.float32).transpose(1, 2, 0).astype(bf)
    )
    W = {
        "f": (np.asarray(W_ih_f, np.float32), np.asarray(W_hh_f, np.float32),
              np.asarray(b_ih_f, np.float32) + np.asarray(b_hh_f, np.float32)),
        "b": (np.asarray(W_ih_b, np.float32), np.asarray(W_hh_b, np.float32),
              np.asarray(b_ih_b, np.float32) + np.asarray(b_hh_b, np.float32)),
    }
    wlinT = np.ascontiguousarray(np.asarray(W_lin, np.float32).T.astype(bf))
    blin_rep = np.broadcast_to(np.asarray(b_lin, np.float32), (B, D_OUT)).copy()
    in_maps = []
    # PyTorch gate blocks in weight rows: i, f, g, o. Our column order is
    # (i, f, o, g) so the three sigmoid gates are contiguous.
    gate_rows = (0, 1, 3, 2)
    for j in range(NC_N):
        m = {"xT": xTs, "wlinT": wlinT, "blin": blin_rep}
        cols = np.concatenate(
            [np.arange(g * H + j * HJ, g * H + (j + 1) * HJ) for g in gate_rows]
        )
        bias_stack = []
        for d in "fb":
            W_ih, W_hh, b_sum = W[d]
            m[f"wihT_{d}"] = np.ascontiguousarray(W_ih.T[:, cols].astype(bf))
            m[f"whhT_{d}"] = np.ascontiguousarray(W_hh.T[:, cols].astype(bf))
            bias_stack.append(np.broadcast_to(b_sum[cols], (B, GJ)))
        m["bias_fb"] = np.ascontiguousarray(np.concatenate(bias_stack, axis=0))
        in_maps.append(m)
    return in_maps


def kernel(**inputs) -> np.ndarray:
    from concourse.bass_utils import run_bass_kernel_spmd

    in_maps = make_in_maps(**inputs)
    nc = build(inputs["x"].shape[1])
    res = run_bass_kernel_spmd(nc, in_maps, core_ids=list(range(NC_N)))
    return res.results[0]["out"]
